# revision 24
# baseline (speedup 1.0000x reference)
"""Trainium2 Bass kernel for DifferentialCrossAttentionLayer.

Math note: softmax(scores - 1.0) == softmax(scores) exactly (shift
invariance along the softmax axis), so
    attn = softmax(s) - sigmoid(lam) * softmax(s - 1) = (1 - sigmoid(lam)) * softmax(s)
The kernel computes standard softmax attention scaled by (1 - sigmoid(lam));
the (1 - sigmoid(lam)) factor is folded into the V projection.

Sharding: 8 cores, each owns 512 query rows (cores 0-3 -> batch 0,
cores 4-7 -> batch 1). No collectives: each core redundantly projects the
full 2048-row K/V of its batch (the extra PE work is far cheaper than a
collective in this system).

Q/K/V projections and Wo run as fp8(e4m3) DoubleRow matmuls (256-deep
contraction per instruction, 4x bf16 throughput); measured end-to-end
these contribute <0.1% extra error because the attention output is small
relative to the residual stream. The FFN stays bf16 (fp8 there costs ~3%
error - the FFN output is ~half of x2). The host pre-transposes and
pre-casts q/k/v to fp8, so there are no device-side input transposes.

Attention is computed in S^T layout: S^T[k, q] per (head, k-tile) is a
single 512-wide matmul (contract = d_head = 128), exp writes P^T directly,
and PV produces attn_out^T via 16 chained 512-wide matmuls. Softmax
denominators: DVE pairwise tree over the 16 k-tiles of P^T, then a
ones-matmul reduces across the 128 k partitions (every output partition
ends up holding den[q], a free partition-broadcast); normalization is
fused into the PSUM->SBUF copy of attn_out^T, which also casts to fp8 as
the Wo operand.

Layer norms batch their statistics across q-tiles (per-tile stats are
emitted as soon as each x tile is ready) and run the two [128, D]
elementwise ops in bf16 to hit the DVE fast path.
"""

import math

import numpy as np
import ml_dtypes

import concourse.bass as bass
import concourse.mybir as mybir
import concourse.tile as tile
from concourse import bacc, bass_utils

F32 = mybir.dt.float32
BF16 = mybir.dt.bfloat16
FP8 = mybir.dt.float8e4
NP_FP8 = ml_dtypes.float8_e4m3
AF = mybir.ActivationFunctionType
ALU = mybir.AluOpType
DR = mybir.MatmulPerfMode.DoubleRow

B = 2
SQ = 2048
SK = 2048
D = 1024
H = 8
DH = 128
FF = 4096
NCORES = 8
R = (B * SQ) // NCORES          # query rows per core = 512
QT = R // 128                   # 4 q-tiles per core
IC = D // 128                   # 8 contraction chunks
KT = SK // 128                  # 16 key tiles
FT = FF // 128                  # 32 ffn-hidden chunks
SCALE = 1.0 / math.sqrt(DH)
LN_EPS = 1e-5


def _build_nc():
    nc = bacc.Bacc("TRN2", target_bir_lowering=False, debug=False,
                   num_devices=NCORES)

    qT8 = nc.dram_tensor("qT8", [D, R], FP8, kind="ExternalInput").ap()
    qbf = nc.dram_tensor("qbf", [R, D], BF16, kind="ExternalInput").ap()
    kT8 = nc.dram_tensor("kT8", [D, SK], FP8, kind="ExternalInput").ap()
    vT8 = nc.dram_tensor("vT8", [D, SK], FP8, kind="ExternalInput").ap()
    Wq = nc.dram_tensor("Wq", [D, D], FP8, kind="ExternalInput").ap()
    Wk = nc.dram_tensor("Wk", [D, D], FP8, kind="ExternalInput").ap()
    Wv = nc.dram_tensor("Wv", [D, D], FP8, kind="ExternalInput").ap()
    Wo = nc.dram_tensor("Wo", [D, D], FP8, kind="ExternalInput").ap()
    lam = nc.dram_tensor("lam", [1, 1], F32, kind="ExternalInput").ap()
    ln1_g = nc.dram_tensor("ln1_g", [1, D], BF16, kind="ExternalInput").ap()
    ln1_b = nc.dram_tensor("ln1_b", [1, D], BF16, kind="ExternalInput").ap()
    ln2_g = nc.dram_tensor("ln2_g", [1, D], BF16, kind="ExternalInput").ap()
    ln2_b = nc.dram_tensor("ln2_b", [1, D], BF16, kind="ExternalInput").ap()
    w1h = nc.dram_tensor("w1h", [D, FF], FP8, kind="ExternalInput").ap()
    w1l = nc.dram_tensor("w1l", [D, FF], FP8, kind="ExternalInput").ap()
    b1s = nc.dram_tensor("b1s", [1, FF], F32, kind="ExternalInput").ap()
    w2 = nc.dram_tensor("w2", [FF, D], BF16, kind="ExternalInput").ap()
    b2 = nc.dram_tensor("b2", [1, D], F32, kind="ExternalInput").ap()
    out = nc.dram_tensor("out", [R, D], BF16, kind="ExternalOutput").ap()

    with tile.TileContext(nc) as tc:
        _emit(nc, tc, locals())
    nc.compile()
    return nc


def _emit(nc, tc, t):
    qT8, qbf, kT8, vT8 = t["qT8"], t["qbf"], t["kT8"], t["vT8"]
    Wq, Wk, Wv, Wo, lam = t["Wq"], t["Wk"], t["Wv"], t["Wo"], t["lam"]
    ln1_g, ln1_b, ln2_g, ln2_b = t["ln1_g"], t["ln1_b"], t["ln2_g"], t["ln2_b"]
    w1h, w1l = t["w1h"], t["w1l"]
    b1s, w2, b2, out = t["b1s"], t["w2"], t["b2"], t["out"]

    g_pool = tc.alloc_tile_pool(name="g", bufs=1)

    # ---- scalar constants ----
    lam_bc = g_pool.tile([128, 1], F32, tag="lam_bc")
    nc.sync.dma_start(lam_bc[:], lam[0:1, :].partition_broadcast(128))
    sig_bc = g_pool.tile([128, 1], F32, tag="sig_bc")
    nc.scalar.activation(sig_bc[:], lam_bc[:], AF.Sigmoid)
    oml_bc = g_pool.tile([128, 1], F32, tag="oml_bc")  # 1 - sigmoid(lam)
    nc.scalar.activation(oml_bc[:], sig_bc[:], AF.Copy, bias=1.0, scale=-1.0)
    # warm the ACT tables (exp/square/sqrt/relu) while ACT is idle
    for fn in (AF.Exp, AF.Square, AF.Sqrt, AF.Relu):
        warm = g_pool.tile([128, 1], F32, tag="warm", name="warm", bufs=4)
        nc.scalar.activation(warm[:], sig_bc[:], fn)

    ones_bf = g_pool.tile([128, 128], BF16, tag="ones_bf")
    nc.vector.memset(ones_bf[:], 1.0)

    # ---- persistent tensors ----
    otn_pool = tc.alloc_tile_pool(name="otn_pool", bufs=1)
    kv_pool = tc.alloc_tile_pool(name="kv_pool", bufs=1)
    khT = kv_pool.tile([128, IC, SK], BF16, tag="khT")   # K-hat^T [d, ic, k]
    vh = kv_pool.tile([128, KT, D], BF16, tag="vh")      # V-hat (x oml) [k, kt, d]
    qhT = kv_pool.tile([128, IC, R], BF16, tag="qhT")    # Q-hat^T x scale [d, ic, q]
    otn = otn_pool.tile([128, H, R], FP8, tag="otn")     # attn_out^T (normalized)

    # Tensors needed right after attention live in otn_pool (its region is
    # disjoint from kv_pool), so their DMA loads can run during the early
    # phases instead of stalling the Wo/FFN1 startup.
    wo_sb = otn_pool.tile([128, IC, D], FP8, tag="wo_sb")
    w1_sb = {}
    for fg in range(2):
        w1_sb[fg] = (
            otn_pool.tile([128, IC, 512], FP8, tag="w1h_sb", name="w1h_sb",
                          bufs=2),
            otn_pool.tile([128, IC, 512], FP8, tag="w1l_sb", name="w1l_sb",
                          bufs=2))
    raw = otn_pool.tile([128, QT, D], BF16, tag="raw")   # query (residual)

    # ================= projections (K, then Q, then V) =================
    with (
        tc.tile_pool(name="proj", bufs=1) as sp,
        tc.tile_pool(name="proj_ps", bufs=2, space="PSUM") as pps,
    ):
        kT_sb = sp.tile([128, IC, SK], FP8, tag="kT_sb")
        vT_sb = sp.tile([128, IC, SK], FP8, tag="vT_sb")
        qT_sb = sp.tile([128, IC, R], FP8, tag="qT_sb")
        wk_sb = sp.tile([128, IC, D], FP8, tag="wk_sb")
        wv_sb = sp.tile([128, IC, D], FP8, tag="wv_sb")
        wq_sb = sp.tile([128, IC, D], FP8, tag="wq_sb")

        nc.scalar.dma_start(wk_sb[:], Wk.rearrange("(i p) d -> p i d", p=128))
        for kh in range(2):
            nc.sync.dma_start(
                kT_sb[:, :, kh * 1024:(kh + 1) * 1024],
                kT8.rearrange("(i p) k -> p i k", p=128)[:, :, kh * 1024:(kh + 1) * 1024])
        nc.sync.dma_start(qT_sb[:], qT8.rearrange("(i p) q -> p i q", p=128))
        nc.scalar.dma_start(wq_sb[:], Wq.rearrange("(i p) d -> p i d", p=128))
        nc.sync.dma_start(vT_sb[:], vT8.rearrange("(i p) k -> p i k", p=128))
        nc.scalar.dma_start(wv_sb[:], Wv.rearrange("(i p) d -> p i d", p=128))
        nc.scalar.dma_start(wo_sb[:], Wo.rearrange("(i p) d -> p i d", p=128))
        for fg in range(2):
            for wsb, wsrc in zip(w1_sb[fg], (w1h, w1l)):
                nc.scalar.dma_start(
                    wsb[:],
                    wsrc.rearrange("(i p) f -> p i f", p=128)[:, :, fg * 512:(fg + 1) * 512])
        nc.sync.dma_start(raw[:], qbf.rearrange("(a p) d -> p a d", p=128))

        # K-hat^T: per dc one [128, 2048] psum, 4 DoubleRow pair-chains
        for dc in range(IC):
            pp = pps.tile([128, 2048], F32, tag="pp", name="pp")
            for tp in range(4):
                for j in range(4):
                    nc.tensor.matmul(
                        pp[:, j * 512:(j + 1) * 512],
                        wk_sb[:, 2 * tp:2 * tp + 2, dc * 128:(dc + 1) * 128],
                        kT_sb[:, 2 * tp:2 * tp + 2, j * 512:(j + 1) * 512],
                        start=(tp == 0), stop=(tp == 3), perf_mode=DR)
            nc.scalar.activation(khT[:, dc, :], pp[:], AF.Copy)

        # Q-hat^T (scaled): two [128, 2048] psums of 4 dc chunks each
        for g in range(2):
            pp = pps.tile([128, 2048], F32, tag="pp", name="pp")
            for tp in range(4):
                for dc4 in range(4):
                    nc.tensor.matmul(
                        pp[:, dc4 * 512:(dc4 + 1) * 512],
                        wq_sb[:, 2 * tp:2 * tp + 2,
                              (4 * g + dc4) * 128:(4 * g + dc4 + 1) * 128],
                        qT_sb[:, 2 * tp:2 * tp + 2, :],
                        start=(tp == 0), stop=(tp == 3), perf_mode=DR)
            nc.scalar.activation(
                qhT[:, 4 * g:4 * g + 4, :],
                pp[:].rearrange("p (a b) -> p a b", a=4), AF.Copy, scale=SCALE)

        # V-hat (x oml): per pair of k-tiles one [128, 2048] psum
        for kt2 in range(KT // 2):
            pp = pps.tile([128, 2048], F32, tag="pp", name="pp")
            for tp in range(4):
                for sub in range(2):
                    for j in range(2):
                        nc.tensor.matmul(
                            pp[:, sub * 1024 + j * 512:sub * 1024 + (j + 1) * 512],
                            vT_sb[:, 2 * tp:2 * tp + 2,
                                  (2 * kt2 + sub) * 128:(2 * kt2 + sub + 1) * 128],
                            wv_sb[:, 2 * tp:2 * tp + 2, j * 512:(j + 1) * 512],
                            start=(tp == 0), stop=(tp == 3), perf_mode=DR)
            nc.vector.tensor_scalar(
                out=vh[:, 2 * kt2:2 * kt2 + 2, :],
                in0=pp[:].rearrange("p (a b) -> p a b", a=2),
                scalar1=oml_bc[:], scalar2=None, op0=ALU.mult)

    # ================= attention (bf16) =================
    with (
        tc.tile_pool(name="attn", bufs=1) as ap,
        tc.tile_pool(name="attn_s", bufs=2, space="PSUM") as sps,
        tc.tile_pool(name="attn_o", bufs=2, space="PSUM") as ops,
    ):
        state = {}

        def emit_head_s(h):
            """S^T matmuls + exp for head h, one 2-k-tile group per step."""
            pts = ap.tile([128, KT, R], BF16, tag="pts", name="pts", bufs=2)
            state[h] = pts
            for g2 in range(KT // 2):
                sp_ = sps.tile([128, 1024], F32, tag="s_ps", name="s_ps")
                for i in range(2):
                    kt = 2 * g2 + i
                    nc.tensor.matmul(
                        sp_[:, i * 512:(i + 1) * 512],
                        khT[:, h, kt * 128:(kt + 1) * 128],
                        qhT[:, h, :],
                        start=True, stop=True)
                nc.scalar.activation(
                    pts[:, 2 * g2:2 * g2 + 2, :],
                    sp_[:].rearrange("p (a b) -> p a b", a=2), AF.Exp)
                yield

        def emit_head_pv(h):
            """den reduction + PV + normalize for head h."""
            pts = state.pop(h)
            tmp8 = ap.tile([128, 8, R], BF16, tag="tmp8", name="tmp8", bufs=1)
            tmp4 = ap.tile([128, 4, R], BF16, tag="tmp4", name="tmp4", bufs=1)
            tmp2 = ap.tile([128, 2, R], BF16, tag="tmp2", name="tmp2", bufs=1)
            partial = ap.tile([128, R], BF16, tag="partial", name="partial", bufs=2)
            nc.vector.tensor_tensor(out=tmp8[:], in0=pts[:, 0:KT:2, :],
                                    in1=pts[:, 1:KT:2, :], op=ALU.add)
            nc.vector.tensor_tensor(out=tmp4[:], in0=tmp8[:, 0:8:2, :],
                                    in1=tmp8[:, 1:8:2, :], op=ALU.add)
            nc.vector.tensor_tensor(out=tmp2[:], in0=tmp4[:, 0:4:2, :],
                                    in1=tmp4[:, 1:4:2, :], op=ALU.add)
            nc.vector.tensor_tensor(out=partial[:], in0=tmp2[:, 0, :],
                                    in1=tmp2[:, 1, :], op=ALU.add)
            den_ps = ops.tile([128, R], F32, tag="den_ps", name="den_ps")
            nc.tensor.matmul(den_ps[:], ones_bf[:], partial[:],
                             start=True, stop=True)
            rec = ap.tile([128, R], F32, tag="rec", name="rec", bufs=2)
            nc.vector.reciprocal(rec[:], den_ps[:])
            ot_ps = ops.tile([128, R], F32, tag="ot_ps", name="ot_ps")
            for kt in range(KT):
                nc.tensor.matmul(
                    ot_ps[:],
                    vh[:, kt, h * 128:(h + 1) * 128],
                    pts[:, kt, :],
                    start=(kt == 0), stop=(kt == KT - 1))
            nc.vector.tensor_tensor(out=otn[:, h, :], in0=ot_ps[:], in1=rec[:],
                                    op=ALU.mult)

        # software pipeline: head h's S/exp interleaves with head h-1's PV
        prev = None
        for h in range(H):
            gen = emit_head_s(h)
            for step in range(KT // 2):
                next(gen, None)
                if step == 3 and prev is not None:
                    emit_head_pv(prev)
            prev = h
        emit_head_pv(prev)

    kv_pool.release()

    post_pool = tc.alloc_tile_pool(name="post", bufs=1)

    def bcast_row(name, src_, dt):
        dst = post_pool.tile([128, D], dt, tag=name, name=name)
        nc.sync.dma_start(dst[:], src_[0:1, :].partition_broadcast(128))
        return dst

    ln1_g_bc = bcast_row("ln1_g_bc", ln1_g, BF16)
    ln1_b_bc = bcast_row("ln1_b_bc", ln1_b, BF16)
    ln2_g_bc = bcast_row("ln2_g_bc", ln2_g, BF16)
    ln2_b_bc = bcast_row("ln2_b_bc", ln2_b, BF16)
    b2_bc = bcast_row("b2_bc", b2, F32)
    b1_sb = post_pool.tile([128, FT], F32, tag="b1_sb")
    nc.sync.dma_start(b1_sb[:], b1s[0, :].rearrange("(c p) -> p c", p=128))

    q_enh_bf = post_pool.tile([128, QT, D], BF16, tag="q_enh_bf")
    q_enh_b2 = post_pool.tile([128, QT, D], BF16, tag="q_enh_b2")
    q_enhT = post_pool.tile([128, IC, R], BF16, tag="q_enhT")
    q_enhT8 = post_pool.tile([128, IC, R], FP8, tag="q_enhT8")
    q_enhTr = post_pool.tile([128, IC, R], FP8, tag="q_enhTr")
    ht = post_pool.tile([128, FT, R], BF16, tag="ht")    # relu(ffn1), ^T

    def ln_stat_tiles(n):
        sums = g_pool.tile([128, n], F32, tag="ln_sums", name="ln")
        ssq = g_pool.tile([128, n], F32, tag="ln_ssq", name="ln")
        return sums, ssq

    def ln_stats(x_t, sums, ssq, i):
        """Per-tile stats (emitted early so they overlap upstream compute)."""
        nc.vector.reduce_sum(sums[:, i:i + 1], x_t, axis=mybir.AxisListType.X)
        sq = g_pool.tile([128, D], BF16, tag="ln_sq", name="ln", bufs=2)
        nc.scalar.activation(sq[:], x_t, AF.Square, accum_out=ssq[:, i:i + 1])

    def ln_finish(x_all, n, sums, ssq, g_bc, b_bc, dst_fn):
        """Normalize n tiles from precomputed sums/ssq; the two [128, D]
        elementwise ops run in bf16 to hit the DVE fast path."""
        mean = g_pool.tile([128, n], F32, tag="ln_mean", name="ln")
        nc.vector.tensor_scalar(out=mean[:], in0=sums[:], scalar1=1.0 / D,
                                scalar2=None, op0=ALU.mult)
        m2 = g_pool.tile([128, n], F32, tag="ln_m2", name="ln")
        nc.vector.tensor_tensor(out=m2[:], in0=mean[:], in1=mean[:], op=ALU.mult)
        v = g_pool.tile([128, n], F32, tag="ln_v", name="ln")
        nc.vector.tensor_scalar(out=v[:], in0=ssq[:], scalar1=1.0 / D,
                                scalar2=LN_EPS, op0=ALU.mult, op1=ALU.add)
        nc.vector.tensor_tensor(out=v[:], in0=v[:], in1=m2[:], op=ALU.subtract)
        std = g_pool.tile([128, n], F32, tag="ln_std", name="ln")
        nc.scalar.activation(std[:], v[:], AF.Sqrt)
        rstd = g_pool.tile([128, n], F32, tag="ln_rstd", name="ln")
        nc.vector.reciprocal(rstd[:], std[:])
        for i in range(n):
            xh = g_pool.tile([128, D], BF16, tag="ln_xh", name="ln", bufs=2)
            nc.vector.tensor_scalar(out=xh[:], in0=x_all[:, i, :],
                                    scalar1=mean[:, i:i + 1],
                                    scalar2=rstd[:, i:i + 1],
                                    op0=ALU.subtract, op1=ALU.mult)
            nc.vector.tensor_tensor(out=xh[:], in0=xh[:], in1=g_bc[:], op=ALU.mult)
            nc.vector.tensor_tensor(out=dst_fn(i), in0=xh[:], in1=b_bc[:],
                                    op=ALU.add)

    # ================= Wo + residual + LN1 =================
    with (
        tc.tile_pool(name="wo", bufs=1) as wp,
        tc.tile_pool(name="wo_ps", bufs=2, space="PSUM") as wps,
    ):
        x1_all = wp.tile([128, QT, D], BF16, tag="x1_all")
        sums1, ssq1 = ln_stat_tiles(QT)
        for qt in range(QT):
            y_ps = wps.tile([128, D], F32, tag="y_ps", name="y_ps")
            for tp in range(4):
                for nd in range(2):
                    nc.tensor.matmul(
                        y_ps[:, nd * 512:(nd + 1) * 512],
                        otn[:, 2 * tp:2 * tp + 2, qt * 128:(qt + 1) * 128],
                        wo_sb[:, 2 * tp:2 * tp + 2, nd * 512:(nd + 1) * 512],
                        start=(tp == 0), stop=(tp == 3), perf_mode=DR)
            nc.vector.tensor_tensor(out=x1_all[:, qt, :], in0=y_ps[:],
                                    in1=raw[:, qt, :], op=ALU.add)
            ln_stats(x1_all[:, qt, :], sums1, ssq1, qt)
        ln_finish(x1_all[:], QT, sums1, ssq1, ln1_g_bc, ln1_b_bc,
                  lambda qt: q_enh_bf[:, qt, :])
        for qt in range(QT):
            nc.sync.dma_start_transpose(
                q_enhT[:, :, qt * 128:(qt + 1) * 128], q_enh_bf[:, qt, :])
            nc.scalar.activation(q_enhT8[:, :, qt * 128:(qt + 1) * 128],
                                 q_enhT[:, :, qt * 128:(qt + 1) * 128], AF.Copy)
            nc.vector.tensor_tensor(
                out=q_enhTr[:, :, qt * 128:(qt + 1) * 128],
                in0=q_enhT[:, :, qt * 128:(qt + 1) * 128],
                in1=q_enhT8[:, :, qt * 128:(qt + 1) * 128], op=ALU.subtract)

    # ================= FFN1 (relu(x @ w1 + b1)) -> ht =================
    with (
        tc.tile_pool(name="ffn1", bufs=1) as fp,
        tc.tile_pool(name="ffn1_ps", bufs=2, space="PSUM") as fps,
    ):
        for fg in range(8):
            if fg >= 2:
                w1_sb[fg] = (
                    fp.tile([128, IC, 512], FP8, tag="w1h_sb", name="w1h_sb",
                            bufs=2),
                    fp.tile([128, IC, 512], FP8, tag="w1l_sb", name="w1l_sb",
                            bufs=2))
                for wsb, wsrc in zip(w1_sb[fg], (w1h, w1l)):
                    nc.scalar.dma_start(
                        wsb[:],
                        wsrc.rearrange("(i p) f -> p i f", p=128)[:, :, fg * 512:(fg + 1) * 512])
            wh, wl = w1_sb[fg]
            ps = fps.tile([128, 2048], F32, tag="hps", name="hps")
            terms = [(wh, q_enhT8), (wh, q_enhTr), (wl, q_enhT8)]
            for ti, (wt, xt) in enumerate(terms):
                for tp in range(4):
                    for fl in range(4):
                        nc.tensor.matmul(
                            ps[:, fl * 512:(fl + 1) * 512],
                            wt[:, 2 * tp:2 * tp + 2, fl * 128:(fl + 1) * 128],
                            xt[:, 2 * tp:2 * tp + 2, :],
                            start=(ti == 0 and tp == 0),
                            stop=(ti == 2 and tp == 3), perf_mode=DR)
            for fl in range(4):
                fc = fg * 4 + fl
                nc.scalar.activation(ht[:, fc, :], ps[:, fl * 512:(fl + 1) * 512],
                                     AF.Relu, bias=b1_sb[:, fc:fc + 1],
                                     scale=0.125)
            del w1_sb[fg]
        # precompute q_enh + b2 for the LN2 residual (idle DVE window)
        for qt in range(QT):
            nc.vector.tensor_tensor(out=q_enh_b2[:, qt, :],
                                    in0=q_enh_bf[:, qt, :],
                                    in1=b2_bc[:], op=ALU.add)

    # ================= FFN2 + residual + LN2 =================
    with (
        tc.tile_pool(name="ffn2", bufs=1) as f2p,
        tc.tile_pool(name="ffn2_ps", bufs=1, space="PSUM") as f2ps,
    ):
        y2 = [f2ps.tile([128, D], F32, tag=f"y2_{qt}", name=f"y2_{qt}")
              for qt in range(QT)]
        for fc in range(FT):
            w2_sb = f2p.tile([128, D], BF16, tag="w2_sb", name="w2_sb", bufs=4)
            nc.scalar.dma_start(w2_sb[:], w2[fc * 128:(fc + 1) * 128, :])
            for qt in range(QT):
                for nd in range(2):
                    nc.tensor.matmul(
                        y2[qt][:, nd * 512:(nd + 1) * 512],
                        ht[:, fc, qt * 128:(qt + 1) * 128],
                        w2_sb[:, nd * 512:(nd + 1) * 512],
                        start=(fc == 0), stop=(fc == FT - 1))
        x2_all = f2p.tile([128, QT, D], BF16, tag="x2_all")
        sums2, ssq2 = ln_stat_tiles(QT)
        for qt in range(QT):
            nc.vector.tensor_tensor(out=x2_all[:, qt, :], in0=y2[qt][:],
                                    in1=q_enh_b2[:, qt, :], op=ALU.add)
            ln_stats(x2_all[:, qt, :], sums2, ssq2, qt)
        ln_finish(x2_all[:], QT, sums2, ssq2, ln2_g_bc, ln2_b_bc,
                  lambda qt: x2_all[:, qt, :])
        for qt in range(QT):
            nc.sync.dma_start(out[qt * 128:(qt + 1) * 128, :], x2_all[:, qt, :])

    post_pool.release()
    otn_pool.release()
    g_pool.release()


_NC_CACHE = None


def _get_nc():
    global _NC_CACHE
    if _NC_CACHE is None:
        _NC_CACHE = _build_nc()
    return _NC_CACHE


def make_in_maps(query, key, value, Wq, Wk, Wv, Wo, lambda_param,
                 ln1_g, ln1_b, ln2_g, ln2_b, ffn_w1, ffn_b1, ffn_w2, ffn_b2):
    f32 = lambda a: np.ascontiguousarray(np.asarray(a, dtype=np.float32))
    bf = lambda a: np.ascontiguousarray(
        np.asarray(a, dtype=np.float32).astype(ml_dtypes.bfloat16))
    fp8 = lambda a: np.ascontiguousarray(
        np.asarray(a, dtype=np.float32).astype(NP_FP8))
    common = {
        "Wq": fp8(Wq), "Wk": fp8(Wk), "Wv": fp8(Wv), "Wo": fp8(Wo),
        "lam": f32(lambda_param).reshape(1, 1),
        "ln1_g": bf(np.asarray(ln1_g, np.float32).reshape(1, D)),
        "ln1_b": bf(np.asarray(ln1_b, np.float32).reshape(1, D)),
        "ln2_g": bf(np.asarray(ln2_g, np.float32).reshape(1, D)),
        "ln2_b": bf(np.asarray(ln2_b, np.float32).reshape(1, D)),
        "b1s": f32(ffn_b1).reshape(1, FF),
        "w2": bf(ffn_w2),
        "b2": f32(ffn_b2).reshape(1, D),
    }
    w1f = np.asarray(ffn_w1, np.float32) * 8.0
    w1h_np = w1f.astype(NP_FP8)
    common["w1h"] = np.ascontiguousarray(w1h_np)
    common["w1l"] = fp8(w1f - w1h_np.astype(np.float32))
    keyT_b = [fp8(np.asarray(key[b], np.float32).T) for b in range(B)]
    valT_b = [fp8(np.asarray(value[b], np.float32).T) for b in range(B)]
    in_maps = []
    for c in range(NCORES):
        b, r0 = c // (NCORES // B), (c % (NCORES // B)) * R
        m = dict(common)
        m["qT8"] = fp8(np.asarray(query[b, r0:r0 + R], np.float32).T)
        m["qbf"] = bf(query[b, r0:r0 + R])
        m["kT8"] = keyT_b[b]
        m["vT8"] = valT_b[b]
        in_maps.append(m)
    return in_maps


def kernel(query, key, value, Wq, Wk, Wv, Wo, lambda_param,
           ln1_g, ln1_b, ln2_g, ln2_b, ffn_w1, ffn_b1, ffn_w2, ffn_b2):
    nc = _get_nc()
    in_maps = make_in_maps(query, key, value, Wq, Wk, Wv, Wo, lambda_param,
                           ln1_g, ln1_b, ln2_g, ln2_b, ffn_w1, ffn_b1,
                           ffn_w2, ffn_b2)
    res = bass_utils.run_bass_kernel_spmd(nc, in_maps, core_ids=list(range(NCORES)))
    outp = np.empty((B, SQ, D), np.float32)
    for c in range(NCORES):
        b, r0 = c // (NCORES // B), (c % (NCORES // B)) * R
        outp[b, r0:r0 + R] = np.asarray(res.results[c]["out"], np.float32)
    return outp
